# revision 3
# baseline (speedup 1.0000x reference)
"""Trainium2 Bass kernel for nn_EnhancedMoELayer (MoE routing, 10 experts, top-2).

Strategy: data-parallel over the 32768-token batch across 8 NeuronCores
(4096 tokens/core). Each core:
  - router: adj_logits = x @ Wr + br + spike bias  (token-major via PE matmul)
  - top-2 selection + combine weights via DVE max8 + exp trick
    (softmax normalization cancels in the top-k renormalization)
  - all-expert FFN (dense baseline): h = relu(x W1 + b1); y = h W2
  - combine: out = b2 + sum_e G[:, e] * y_e
Host side only reshapes/shards numpy arrays and concatenates results.
"""

import numpy as np

import concourse.bass as bass
import concourse.mybir as mybir
import concourse.tile as tile
from concourse import bacc
from concourse.bass_utils import run_bass_kernel_spmd

N_CORES = 8
B, D_IN, HIDDEN, D_OUT = 32768, 512, 1024, 256
E = 10  # total experts (8 + 2 spike)
TC = B // N_CORES  # tokens per core
CH = 512  # tokens per FFN chunk
N_CH = TC // CH
N_SUB = TC // 128  # 128-token subtiles per core

f32 = mybir.dt.float32
AF = mybir.ActivationFunctionType
ALU = mybir.AluOpType


def build_dense():
    nc = bacc.Bacc("TRN2", target_bir_lowering=False, debug=False)
    xT = nc.dram_tensor("xT", [D_IN, TC], f32, kind="ExternalInput").ap()
    spike = nc.dram_tensor("spike", [TC, 16], f32, kind="ExternalInput").ap()
    Wr = nc.dram_tensor("Wr", [D_IN, E], f32, kind="ExternalInput").ap()
    br = nc.dram_tensor("br", [1, E], f32, kind="ExternalInput").ap()
    W1 = nc.dram_tensor("W1", [E, D_IN, HIDDEN], f32, kind="ExternalInput").ap()
    b1r = nc.dram_tensor("b1r", [128, E * 8], f32, kind="ExternalInput").ap()
    W2 = nc.dram_tensor("W2", [E, HIDDEN, D_OUT], f32, kind="ExternalInput").ap()
    b2bc_d = nc.dram_tensor("b2bc", [128, D_OUT], f32, kind="ExternalInput").ap()
    out = nc.dram_tensor("out", [TC, D_OUT], f32, kind="ExternalOutput").ap()

    with tile.TileContext(nc) as tc:
        with (
            tc.tile_pool(name="const", bufs=1) as constp,
            tc.tile_pool(name="xres", bufs=1) as xresp,
            tc.tile_pool(name="accp", bufs=1) as accp,
            tc.tile_pool(name="small", bufs=4) as smp,
        ):
            # constants
            wr_sb = constp.tile([128, 4, E], f32)
            for k in range(4):
                nc.sync.dma_start(out=wr_sb[:, k, :], in_=Wr[k * 128 : (k + 1) * 128, :])
            br_sb = constp.tile([1, E], f32)
            nc.sync.dma_start(out=br_sb[:], in_=br[:])
            b1_sb = constp.tile([128, E * 8], f32)
            nc.sync.dma_start(out=b1_sb[:], in_=b1r[:])
            b2bc = constp.tile([128, D_OUT], f32)
            nc.sync.dma_start(out=b2bc[:], in_=b2bc_d[:])
            ones_row = constp.tile([1, 128], f32)
            nc.vector.memset(ones_row[:], 1.0)

            # resident xT: 4 d-tiles of [128, TC]
            xt = []
            for k in range(4):
                t = xresp.tile([128, TC], f32, tag=f"xt{k}")
                nc.sync.dma_start(out=t[:], in_=xT[k * 128 : (k + 1) * 128, :])
                xt.append(t)

            # gates and accumulators for all subtiles
            G_all = accp.tile([128, N_SUB, E], f32)
            acc_all = accp.tile([128, N_SUB, D_OUT], f32)

            # ---------------- router ----------------
            with tc.tile_pool(name="psr", bufs=2, space="PSUM") as psr:
                for s in range(N_SUB):
                    adj = psr.tile([128, E], f32)
                    for k in range(4):
                        nc.tensor.matmul(
                            adj[:],
                            lhsT=xt[k][:, s * 128 : (s + 1) * 128],
                            rhs=wr_sb[:, k, :],
                            start=(k == 0),
                            stop=False,
                        )
                    # + br (rank-1 broadcast over tokens)
                    nc.tensor.matmul(
                        adj[:], lhsT=ones_row[:], rhs=br_sb[:], start=False, stop=True
                    )
                    sp = smp.tile([128, 16], f32, tag="sp")
                    nc.sync.dma_start(out=sp[:], in_=spike[s * 128 : (s + 1) * 128, :])
                    avg = smp.tile([128, 1], f32, tag="avg")
                    nc.vector.reduce_sum(avg[:], sp[:], axis=mybir.AxisListType.X)
                    nc.vector.tensor_scalar_mul(avg[:], avg[:], 1.0 / 16.0)
                    A = smp.tile([128, E], f32, tag="A")
                    nc.vector.tensor_copy(A[:], adj[:])
                    nc.vector.tensor_scalar_add(A[:, 8:10], A[:, 8:10], avg[:])
                    # top-2 + gates
                    M8 = smp.tile([128, 8], f32, tag="M8")
                    nc.vector.max(M8[:], A[:])
                    negm1 = smp.tile([128, 1], f32, tag="negm1")
                    nc.vector.tensor_scalar_mul(negm1[:], M8[:, 0:1], -1.0)
                    S = smp.tile([128, E], f32, tag="S")
                    nc.scalar.activation(S[:], A[:], AF.Exp, bias=negm1[:], scale=1.0)
                    Mk = smp.tile([128, E], f32, tag="Mk")
                    nc.vector.tensor_scalar(
                        Mk[:], A[:], M8[:, 1:2], None, op0=ALU.is_ge
                    )
                    Sg = smp.tile([128, E], f32, tag="Sg")
                    nc.vector.tensor_mul(Sg[:], S[:], Mk[:])
                    r = smp.tile([128, 1], f32, tag="r")
                    nc.vector.reduce_sum(r[:], Sg[:], axis=mybir.AxisListType.X)
                    rr = smp.tile([128, 1], f32, tag="rr")
                    nc.vector.reciprocal(rr[:], r[:])
                    nc.vector.tensor_scalar_mul(G_all[:, s, :], Sg[:], rr[:])
                    # init accumulator with b2
                    nc.vector.tensor_copy(acc_all[:, s, :], b2bc[:])

            # ---------------- dense all-expert FFN ----------------
            with (
                tc.tile_pool(name="w1p", bufs=8) as w1p,
                tc.tile_pool(name="w2p", bufs=2) as w2p,
                tc.tile_pool(name="hp", bufs=8) as hp,
                tc.tile_pool(name="gyp", bufs=4) as gyp,
                tc.tile_pool(name="psh", bufs=4, space="PSUM") as psh,
                tc.tile_pool(name="psy", bufs=4, space="PSUM") as psy,
            ):
                for e in range(E):
                    w1t = []
                    for k in range(4):
                        t = w1p.tile([128, HIDDEN], f32, tag="w1")
                        nc.sync.dma_start(out=t[:], in_=W1[e, k * 128 : (k + 1) * 128, :])
                        w1t.append(t)
                    w2t = w2p.tile([128, 8, D_OUT], f32, tag="w2")
                    for kk in range(8):
                        nc.sync.dma_start(
                            out=w2t[:, kk, :], in_=W2[e, kk * 128 : (kk + 1) * 128, :]
                        )
                    for c in range(N_CH):
                        y_ps = [psy.tile([128, D_OUT], f32, tag="y", name=f"y_ps{i}") for i in range(4)]
                        for hh in range(2):
                            h_ps = [psh.tile([128, CH], f32, tag="h", name=f"h_ps{m}") for m in range(4)]
                            for m in range(4):
                                hcol = (hh * 4 + m) * 128
                                for k in range(4):
                                    nc.tensor.matmul(
                                        h_ps[m][:],
                                        lhsT=w1t[k][:, hcol : hcol + 128],
                                        rhs=xt[k][:, c * CH : (c + 1) * CH],
                                        start=(k == 0),
                                        stop=(k == 3),
                                    )
                            h_sb = [hp.tile([128, CH], f32, tag="hsb", name=f"h_sb{m}") for m in range(4)]
                            for m in range(4):
                                nc.scalar.activation(
                                    h_sb[m][:],
                                    h_ps[m][:],
                                    AF.Relu,
                                    bias=b1_sb[:, e * 8 + hh * 4 + m : e * 8 + hh * 4 + m + 1],
                                    scale=1.0,
                                )
                            for i in range(4):
                                for kk in range(4):
                                    nc.tensor.matmul(
                                        y_ps[i][:],
                                        lhsT=h_sb[kk][:, i * 128 : (i + 1) * 128],
                                        rhs=w2t[:, hh * 4 + kk, :],
                                        start=(hh == 0 and kk == 0),
                                        stop=(hh == 1 and kk == 3),
                                    )
                        for i in range(4):
                            s = c * 4 + i
                            gy = gyp.tile([128, D_OUT], f32, tag="gy")
                            nc.scalar.activation(
                                gy[:], y_ps[i][:], AF.Copy, bias=0.0,
                                scale=G_all[:, s, e : e + 1],
                            )
                            nc.vector.tensor_add(
                                acc_all[:, s, :], acc_all[:, s, :], gy[:]
                            )

            for s in range(N_SUB):
                nc.sync.dma_start(
                    out=out[s * 128 : (s + 1) * 128, :], in_=acc_all[:, s, :]
                )

    nc.compile()
    return nc


_NC_CACHE = {}


def _get_nc():
    if "nc" not in _NC_CACHE:
        _NC_CACHE["nc"] = build_dense()
    return _NC_CACHE["nc"]


def _prep_in_maps(inputs):
    x = np.asarray(inputs["x"], dtype=np.float32)
    spike = np.asarray(inputs["spike_indicators"], dtype=np.float32)
    Wr = np.asarray(inputs["Wr"], dtype=np.float32)
    br = np.asarray(inputs["br"], dtype=np.float32)
    W1 = np.asarray(inputs["W1"], dtype=np.float32)
    b1 = np.asarray(inputs["b1"], dtype=np.float32)
    W2 = np.asarray(inputs["W2"], dtype=np.float32)
    b2 = np.asarray(inputs["b2"], dtype=np.float32)

    b1r = np.ascontiguousarray(
        b1.reshape(E, 8, 128).transpose(2, 0, 1).reshape(128, E * 8)
    )
    b2bc = np.ascontiguousarray(np.tile(b2[None, :], (128, 1)))
    shared = {
        "Wr": np.ascontiguousarray(Wr),
        "br": np.ascontiguousarray(br[None, :]),
        "W1": np.ascontiguousarray(W1),
        "b1r": b1r,
        "W2": np.ascontiguousarray(W2),
        "b2bc": b2bc,
    }
    in_maps = []
    for c in range(N_CORES):
        xs = x[c * TC : (c + 1) * TC]
        in_maps.append(
            {
                "xT": np.ascontiguousarray(xs.T),
                "spike": np.ascontiguousarray(spike[c * TC : (c + 1) * TC]),
                **shared,
            }
        )
    return in_maps


def kernel(**inputs) -> np.ndarray:
    in_maps = _prep_in_maps(inputs)
    nc = _get_nc()
    res = run_bass_kernel_spmd(nc, in_maps, core_ids=list(range(N_CORES)))
    out = np.concatenate([res.results[c]["out"] for c in range(N_CORES)], axis=0)
    return out.astype(np.float32)


def run_traced(**inputs):
    in_maps = _prep_in_maps(inputs)
    nc = _get_nc()
    return run_bass_kernel_spmd(
        nc, in_maps, core_ids=list(range(N_CORES)), trace=True
    )


# revision 4
# speedup vs baseline: 3.3239x; 3.3239x over previous
"""Trainium2 Bass kernel for nn_EnhancedMoELayer (MoE routing, 10 experts, top-2).

Strategy: data-parallel over the 32768-token batch across 8 NeuronCores
(4096 tokens/core). Each core:
  - router: adj_logits = x @ Wr + br + spike bias  (token-major via PE matmul)
  - top-2 selection + combine weights via DVE max8 + exp trick
    (softmax normalization cancels in the top-k renormalization)
  - all-expert FFN (dense baseline): h = relu(x W1 + b1); y = h W2
  - combine: out = b2 + sum_e G[:, e] * y_e
Host side only reshapes/shards numpy arrays and concatenates results.
"""

import numpy as np

import concourse.bass as bass
import concourse.mybir as mybir
import concourse.tile as tile
from concourse import bacc
from concourse.bass_utils import run_bass_kernel_spmd

N_CORES = 8
B, D_IN, HIDDEN, D_OUT = 32768, 512, 1024, 256
E = 10  # total experts (8 + 2 spike)
TC = B // N_CORES  # tokens per core
CH = 512  # tokens per FFN chunk
N_CH = TC // CH
N_SUB = TC // 128  # 128-token subtiles per core

f32 = mybir.dt.float32
bf16 = mybir.dt.bfloat16
AF = mybir.ActivationFunctionType
ALU = mybir.AluOpType


def build_dense():
    nc = bacc.Bacc("TRN2", target_bir_lowering=False, debug=False)
    xT = nc.dram_tensor("xT", [D_IN, TC], f32, kind="ExternalInput").ap()
    spike = nc.dram_tensor("spike", [TC, 16], f32, kind="ExternalInput").ap()
    Wr = nc.dram_tensor("Wr", [D_IN, E], f32, kind="ExternalInput").ap()
    br = nc.dram_tensor("br", [1, E], f32, kind="ExternalInput").ap()
    W1 = nc.dram_tensor("W1", [E, D_IN, HIDDEN], bf16, kind="ExternalInput").ap()
    b1r = nc.dram_tensor("b1r", [128, E * 8], f32, kind="ExternalInput").ap()
    W2 = nc.dram_tensor("W2", [E, HIDDEN, D_OUT], bf16, kind="ExternalInput").ap()
    b2bc_d = nc.dram_tensor("b2bc", [128, D_OUT], f32, kind="ExternalInput").ap()
    out = nc.dram_tensor("out", [TC, D_OUT], f32, kind="ExternalOutput").ap()

    with tile.TileContext(nc) as tc:
        with (
            tc.tile_pool(name="const", bufs=1) as constp,
            tc.tile_pool(name="xres", bufs=1) as xresp,
            tc.tile_pool(name="accp", bufs=1) as accp,
            tc.tile_pool(name="small", bufs=4) as smp,
        ):
            # constants
            wr_sb = constp.tile([128, 4, E], f32)
            for k in range(4):
                nc.sync.dma_start(out=wr_sb[:, k, :], in_=Wr[k * 128 : (k + 1) * 128, :])
            br_sb = constp.tile([1, E], f32)
            nc.sync.dma_start(out=br_sb[:], in_=br[:])
            b1_sb = constp.tile([128, E * 8], f32)
            nc.sync.dma_start(out=b1_sb[:], in_=b1r[:])
            b2bc = constp.tile([128, D_OUT], f32)
            nc.sync.dma_start(out=b2bc[:], in_=b2bc_d[:])
            ones_row = constp.tile([1, 128], f32)
            nc.vector.memset(ones_row[:], 1.0)

            # resident xT: 4 d-tiles of [128, TC]
            xt = []
            for k in range(4):
                t = xresp.tile([128, TC], f32, tag=f"xt{k}")
                nc.sync.dma_start(out=t[:], in_=xT[k * 128 : (k + 1) * 128, :])
                xt.append(t)
            xtb = []
            for k in range(4):
                tb = xresp.tile([128, TC], bf16, tag=f"xtb{k}", name=f"xtb{k}")
                nc.vector.tensor_copy(tb[:], xt[k][:])
                xtb.append(tb)

            # gates and accumulators for all subtiles
            G_all = accp.tile([128, N_SUB, E], f32)
            acc_all = accp.tile([128, N_SUB, D_OUT], f32)

            # ---------------- router ----------------
            with tc.tile_pool(name="psr", bufs=2, space="PSUM") as psr:
                for s in range(N_SUB):
                    adj = psr.tile([128, E], f32)
                    for k in range(4):
                        nc.tensor.matmul(
                            adj[:],
                            lhsT=xt[k][:, s * 128 : (s + 1) * 128],
                            rhs=wr_sb[:, k, :],
                            start=(k == 0),
                            stop=False,
                        )
                    # + br (rank-1 broadcast over tokens)
                    nc.tensor.matmul(
                        adj[:], lhsT=ones_row[:], rhs=br_sb[:], start=False, stop=True
                    )
                    sp = smp.tile([128, 16], f32, tag="sp")
                    nc.sync.dma_start(out=sp[:], in_=spike[s * 128 : (s + 1) * 128, :])
                    avg = smp.tile([128, 1], f32, tag="avg")
                    nc.vector.reduce_sum(avg[:], sp[:], axis=mybir.AxisListType.X)
                    nc.vector.tensor_scalar_mul(avg[:], avg[:], 1.0 / 16.0)
                    A = smp.tile([128, E], f32, tag="A")
                    nc.vector.tensor_copy(A[:], adj[:])
                    nc.vector.tensor_scalar_add(A[:, 8:10], A[:, 8:10], avg[:])
                    # top-2 + gates
                    M8 = smp.tile([128, 8], f32, tag="M8")
                    nc.vector.max(M8[:], A[:])
                    negm1 = smp.tile([128, 1], f32, tag="negm1")
                    nc.vector.tensor_scalar_mul(negm1[:], M8[:, 0:1], -1.0)
                    S = smp.tile([128, E], f32, tag="S")
                    nc.scalar.activation(S[:], A[:], AF.Exp, bias=negm1[:], scale=1.0)
                    Mk = smp.tile([128, E], f32, tag="Mk")
                    nc.vector.tensor_scalar(
                        Mk[:], A[:], M8[:, 1:2], None, op0=ALU.is_ge
                    )
                    Sg = smp.tile([128, E], f32, tag="Sg")
                    nc.vector.tensor_mul(Sg[:], S[:], Mk[:])
                    r = smp.tile([128, 1], f32, tag="r")
                    nc.vector.reduce_sum(r[:], Sg[:], axis=mybir.AxisListType.X)
                    rr = smp.tile([128, 1], f32, tag="rr")
                    nc.vector.reciprocal(rr[:], r[:])
                    nc.vector.tensor_scalar_mul(G_all[:, s, :], Sg[:], rr[:])
                    # init accumulator with b2
                    nc.vector.tensor_copy(acc_all[:, s, :], b2bc[:])

            # ---------------- dense all-expert FFN ----------------
            with (
                tc.tile_pool(name="w1p", bufs=8) as w1p,
                tc.tile_pool(name="w2p", bufs=2) as w2p,
                tc.tile_pool(name="hp", bufs=8) as hp,
                tc.tile_pool(name="gyp", bufs=4) as gyp,
                tc.tile_pool(name="psh", bufs=4, space="PSUM") as psh,
                tc.tile_pool(name="psy", bufs=4, space="PSUM") as psy,
            ):
                for e in range(E):
                    w1t = []
                    for k in range(4):
                        t = w1p.tile([128, HIDDEN], bf16, tag="w1")
                        nc.sync.dma_start(out=t[:], in_=W1[e, k * 128 : (k + 1) * 128, :])
                        w1t.append(t)
                    w2t = w2p.tile([128, 8, D_OUT], bf16, tag="w2")
                    for kk in range(8):
                        nc.sync.dma_start(
                            out=w2t[:, kk, :], in_=W2[e, kk * 128 : (kk + 1) * 128, :]
                        )
                    for c in range(N_CH):
                        y_ps = [psy.tile([128, D_OUT], f32, tag="y", name=f"y_ps{i}") for i in range(4)]
                        for hh in range(2):
                            h_ps = [psh.tile([128, CH], f32, tag="h", name=f"h_ps{m}") for m in range(4)]
                            for m in range(4):
                                hcol = (hh * 4 + m) * 128
                                for k in range(4):
                                    nc.tensor.matmul(
                                        h_ps[m][:],
                                        lhsT=w1t[k][:, hcol : hcol + 128],
                                        rhs=xtb[k][:, c * CH : (c + 1) * CH],
                                        start=(k == 0),
                                        stop=(k == 3),
                                    )
                            h_sb = [hp.tile([128, CH], bf16, tag="hsb", name=f"h_sb{m}") for m in range(4)]
                            for m in range(4):
                                nc.scalar.activation(
                                    h_sb[m][:],
                                    h_ps[m][:],
                                    AF.Relu,
                                    bias=b1_sb[:, e * 8 + hh * 4 + m : e * 8 + hh * 4 + m + 1],
                                    scale=1.0,
                                )
                            for i in range(4):
                                for kk in range(4):
                                    nc.tensor.matmul(
                                        y_ps[i][:],
                                        lhsT=h_sb[kk][:, i * 128 : (i + 1) * 128],
                                        rhs=w2t[:, hh * 4 + kk, :],
                                        start=(hh == 0 and kk == 0),
                                        stop=(hh == 1 and kk == 3),
                                    )
                        for i in range(4):
                            s = c * 4 + i
                            gy = gyp.tile([128, D_OUT], f32, tag="gy")
                            nc.scalar.activation(
                                gy[:], y_ps[i][:], AF.Copy, bias=0.0,
                                scale=G_all[:, s, e : e + 1],
                            )
                            nc.vector.tensor_add(
                                acc_all[:, s, :], acc_all[:, s, :], gy[:]
                            )

            for s in range(N_SUB):
                nc.sync.dma_start(
                    out=out[s * 128 : (s + 1) * 128, :], in_=acc_all[:, s, :]
                )

    nc.compile()
    return nc


_NC_CACHE = {}


def _get_nc():
    if "nc" not in _NC_CACHE:
        _NC_CACHE["nc"] = build_dense()
    return _NC_CACHE["nc"]


def _prep_in_maps(inputs):
    x = np.asarray(inputs["x"], dtype=np.float32)
    spike = np.asarray(inputs["spike_indicators"], dtype=np.float32)
    Wr = np.asarray(inputs["Wr"], dtype=np.float32)
    br = np.asarray(inputs["br"], dtype=np.float32)
    W1 = np.asarray(inputs["W1"], dtype=np.float32)
    b1 = np.asarray(inputs["b1"], dtype=np.float32)
    W2 = np.asarray(inputs["W2"], dtype=np.float32)
    b2 = np.asarray(inputs["b2"], dtype=np.float32)

    b1r = np.ascontiguousarray(
        b1.reshape(E, 8, 128).transpose(2, 0, 1).reshape(128, E * 8)
    )
    b2bc = np.ascontiguousarray(np.tile(b2[None, :], (128, 1)))
    import ml_dtypes

    shared = {
        "Wr": np.ascontiguousarray(Wr),
        "br": np.ascontiguousarray(br[None, :]),
        "W1": np.ascontiguousarray(W1).astype(ml_dtypes.bfloat16),
        "b1r": b1r,
        "W2": np.ascontiguousarray(W2).astype(ml_dtypes.bfloat16),
        "b2bc": b2bc,
    }
    in_maps = []
    for c in range(N_CORES):
        xs = x[c * TC : (c + 1) * TC]
        in_maps.append(
            {
                "xT": np.ascontiguousarray(xs.T),
                "spike": np.ascontiguousarray(spike[c * TC : (c + 1) * TC]),
                **shared,
            }
        )
    return in_maps


def kernel(**inputs) -> np.ndarray:
    in_maps = _prep_in_maps(inputs)
    nc = _get_nc()
    res = run_bass_kernel_spmd(nc, in_maps, core_ids=list(range(N_CORES)))
    out = np.concatenate([res.results[c]["out"] for c in range(N_CORES)], axis=0)
    return out.astype(np.float32)


def run_traced(**inputs):
    in_maps = _prep_in_maps(inputs)
    nc = _get_nc()
    return run_bass_kernel_spmd(
        nc, in_maps, core_ids=list(range(N_CORES)), trace=True
    )
